# revision 39
# baseline (speedup 1.0000x reference)
"""MemoryBank kernel for 8x TRN2 NeuronCores (SPMD, batch-parallel), v3.

Algebraic restructure (exact in real arithmetic):
    scores   = x @ (memory @ key_w).T            # fold key proj into 64-slot table
    gate_x   = x @ gate_w[:, :D].T               # 65th column of the folded table
    attn     = softmax(scores / 0.1)
    retrieved= attn @ (memory @ value_w.T)       # fold value proj into table
    ret_gate = attn @ (memory @ value_w.T @ gate_w[:, D:].T)
    gate     = sigmoid(gate_x + ret_gate + b)
    out      = gate * x + (1 - gate) * retrieved

Design (v3):
  * S matmul in split-bf16 (x = xh+xl, G = Gh+Gl; S = x@[Gh|gxh] + xh@Gl,
    dropping the 2^-18 xl@Gl term): full-rate bf16 instead of 4-cycle/row
    fp32 LOW_HIGH. Logit error ~2e-3 against sharp-softmax tolerance ~0.1.
  * Slot-major softmax: per-token max via gpsimd partition_all_reduce
    (result pre-broadcast over the 64 slot partitions), exp on ACT with the
    only table load of the kernel (sigmoid is computed as 1/(1+exp(-z))).
  * Per-token scalars (1/sum, gate, cp=(g-1)/sum) run token-major [128,4]
    via tiny f32r PE transposes (single-instruction, unlike fp32 LOW_HIGH),
    then broadcast back with 1-row f32r ones-matmuls.
  * Combine out = g*x + Ep@Wv: DVE does all 16 muls (bf16*fp32->bf16) and
    4 PSUM-direct adds; ACT drains 12 R-tiles to SBUF bf16 for gpsimd adds
    (gpsimd cannot touch PSUM).
  * bf16 output: 48MiB HBM traffic per core (~140us roofline at 358GB/s).
  * 3-deep software pipeline: combine(t) | stats_back(t+1) | stats_front(t+2)
    | S(t+3) | DMA(t+3) so the ~2-cadence stats latency never blocks combine.
"""

from contextlib import ExitStack

import numpy as np
import ml_dtypes

import concourse.bass as bass
import concourse.tile as tile
from concourse import bacc
from concourse import bass_isa
from concourse import mybir
from concourse.bass import ts
from concourse.bass_utils import run_bass_kernel_spmd

F32 = mybir.dt.float32
F32R = mybir.dt.float32r
BF16 = mybir.dt.bfloat16
AX = mybir.AxisListType
ALU = mybir.AluOpType
ACTF = mybir.ActivationFunctionType

B = 8
L = 4096
DIM = 2048
NSLOT = 64
NCH = DIM // 128  # 16 dim chunks
TOK = 512  # tokens per tile
NT = L // TOK  # 8 tiles per core
NQ = TOK // 128  # 4 token quarters per tile

N_DVE_ADD = 6  # combine adds on DVE (PSUM-direct); rest PE ident-add + drain


def _build(gate_b: float) -> bass.Bass:
    nc = bacc.Bacc("TRN2", target_bir_lowering=False, debug=False)

    xh = nc.dram_tensor("xh", [DIM, L], BF16, kind="ExternalInput").ap()
    xl = nc.dram_tensor("xl", [DIM, L], BF16, kind="ExternalInput").ap()
    G1 = nc.dram_tensor("G1", [DIM, NSLOT + 1], BF16, kind="ExternalInput").ap()
    G2 = nc.dram_tensor("G2", [DIM, NSLOT + 1], BF16, kind="ExternalInput").ap()
    Wv = nc.dram_tensor("Wv", [NSLOT, DIM], BF16, kind="ExternalInput").ap()
    auxsg = nc.dram_tensor("auxsg", [NSLOT, 2], F32R, kind="ExternalInput").ap()
    auxbc = nc.dram_tensor("auxbc", [1, 128], F32R, kind="ExternalInput").ap()
    identT = nc.dram_tensor("identT", [128, 128], F32, kind="ExternalInput").ap()
    identA = nc.dram_tensor("identA", [128, 128], BF16, kind="ExternalInput").ap()
    outT = nc.dram_tensor("outT", [DIM, L], BF16, kind="ExternalOutput").ap()

    # dim d = c*128 + p  (chunk-major split; consistent everywhere)
    xh_v = xh.rearrange("(c p) t -> p c t", p=128)
    xl_v = xl.rearrange("(c p) t -> p c t", p=128)
    G1_v = G1.rearrange("(c p) m -> p c m", p=128)
    G2_v = G2.rearrange("(c p) m -> p c m", p=128)
    Wv_v = Wv.rearrange("n (c q) -> n c q", q=128)
    outT_v = outT.rearrange("(c p) t -> p c t", p=128)

    with tile.TileContext(nc) as tc, ExitStack() as ctx:
        consts = ctx.enter_context(tc.tile_pool(name="consts", bufs=1))
        xhpool = ctx.enter_context(tc.tile_pool(name="xhpool", bufs=5))
        xlpool = ctx.enter_context(tc.tile_pool(name="xlpool", bufs=2))
        opool = ctx.enter_context(tc.tile_pool(name="opool", bufs=2))
        work = ctx.enter_context(tc.tile_pool(name="work", bufs=3))
        small = ctx.enter_context(tc.tile_pool(name="small", bufs=2))
        xgpool = ctx.enter_context(tc.tile_pool(name="xgpool", bufs=10))
        xg2pool = ctx.enter_context(tc.tile_pool(name="xg2pool", bufs=6))
        psS = ctx.enter_context(tc.tile_pool(name="psS", bufs=2, space="PSUM"))
        psR = ctx.enter_context(tc.tile_pool(name="psR", bufs=3, space="PSUM"))
        # single-buffered aux banks; same-shape tags reuse a bank across the
        # serial stats chain (WAR hazards order them)
        psX = ctx.enter_context(tc.tile_pool(name="psX", bufs=1, space="PSUM"))

        G1_sb = consts.tile([128, NCH, NSLOT + 1], BF16)
        nc.sync.dma_start(out=G1_sb, in_=G1_v)
        G2_sb = consts.tile([128, NCH, NSLOT + 1], BF16)
        nc.sync.dma_start(out=G2_sb, in_=G2_v)
        Wv_sb = consts.tile([NSLOT, NCH, 128], BF16)
        nc.sync.dma_start(out=Wv_sb, in_=Wv_v)
        auxsg_sb = consts.tile([NSLOT, 2], F32R)
        nc.sync.dma_start(out=auxsg_sb, in_=auxsg)
        auxbc_sb = consts.tile([1, 128], F32R)
        nc.sync.dma_start(out=auxbc_sb, in_=auxbc)
        identT_sb = consts.tile([128, 128], F32)
        nc.sync.dma_start(out=identT_sb, in_=identT)
        identA_sb = consts.tile([128, 128], BF16)
        nc.sync.dma_start(out=identA_sb, in_=identA)

        def phase_dma(t):
            """prefetch x^T hi/lo tiles (3 tiles ahead of use)."""
            xh_sb = xhpool.tile([128, NCH, TOK], BF16, tag="xh_sb")
            nc.sync.dma_start(out=xh_sb, in_=xh_v[:, :, ts(t, TOK)])
            xl_sb = xlpool.tile([128, NCH, TOK], BF16, tag="xl_sb")
            nc.sync.dma_start(out=xl_sb, in_=xl_v[:, :, ts(t, TOK)])
            return {"xh": xh_sb, "xl": xl_sb}

        def s_phase(t, st):
            """split-bf16 score matmuls + PSUM->SBUF copy.

            S = x@[Gh|gxh] + xh@[Gl|0]  (rows 0..63 slots, row 64 gate x-part)
            """
            xh_sb, xl_sb = st["xh"], st["xl"]
            S_big = psS.tile([128, TOK], F32, tag="S")
            S_ps = S_big[0 : NSLOT + 1, :]
            for c in range(NCH):
                nc.tensor.matmul(
                    S_ps, G1_sb[:, c, :], xh_sb[:, c, :],
                    start=(c == 0), stop=False,
                )
                nc.tensor.matmul(
                    S_ps, G1_sb[:, c, :], xl_sb[:, c, :],
                    start=False, stop=False,
                )
                nc.tensor.matmul(
                    S_ps, G2_sb[:, c, :], xh_sb[:, c, :],
                    start=False, stop=(c == NCH - 1),
                )
            S_sb = work.tile([NSLOT + 1, TOK], F32, tag="S_sb")
            nc.scalar.copy(S_sb, S_ps)
            st["S_sb"] = S_sb

        def stats_front(t, st):
            """max (gpsimd, pre-broadcast), exp, slot sums, token-major stage."""
            S_sb = st["S_sb"]
            mxa = small.tile([NSLOT, TOK], F32, tag="mxa")
            nc.gpsimd.partition_all_reduce(
                mxa, S_sb[0:NSLOT, :], channels=NSLOT,
                reduce_op=bass_isa.ReduceOp.max,
            )
            Ssub = work.tile([NSLOT, TOK], F32, tag="Ssub")
            nc.vector.tensor_sub(Ssub, S_sb[0:NSLOT, :], mxa)
            # F32R so the sums/gvd matmul may consume it directly
            E = work.tile([NSLOT, TOK], F32R, tag="E")
            nc.scalar.activation(E, Ssub, func=ACTF.Exp, bias=0.0, scale=10.0)
            st["E"] = E
            sg_ps = psX.tile([2, NQ, 128], F32, tag="bc2")
            sg_flat = sg_ps.rearrange("p a b -> p (a b)")
            nc.tensor.matmul(sg_flat, auxsg_sb, E, start=True, stop=True)
            sgx_sb = work.tile([2, TOK], F32, tag="sgx_sb")
            nc.scalar.copy(sgx_sb, sg_flat)
            gx_sb = work.tile([1, TOK], F32, tag="gx_sb")
            nc.scalar.copy(gx_sb, S_sb[NSLOT : NSLOT + 1, :])
            tok_ps = psX.tile([128, NQ, 3], F32, tag="tok")
            for q in range(NQ):
                nc.tensor.transpose(
                    tok_ps[:, q, 0:2], sgx_sb[:, ts(q, 128)],
                    identT_sb[0:2, 0:2],
                )
                nc.tensor.transpose(
                    tok_ps[:, q, 2:3], gx_sb[:, ts(q, 128)],
                    identT_sb[0:1, 0:1],
                )
            st["tok"] = tok_ps

        def stats_chain(t, st):
            """token-major scalar chain (DVE+ACT only; issued at iter start
            so the PE's transposes never wait on it mid-tile)."""
            tok_ps = st["tok"]
            r = small.tile([128, NQ], F32, tag="r")
            nc.vector.reciprocal(r, tok_ps[:, :, 0])
            t1 = small.tile([128, NQ], F32, tag="t1")
            nc.vector.tensor_mul(t1, tok_ps[:, :, 1], r)
            gl = small.tile([128, NQ], F32, tag="gl")
            nc.vector.tensor_add(gl, t1, tok_ps[:, :, 2])
            # sigmoid(z) = 1/(1+exp(-z)) on the Exp table (no table thrash)
            en = small.tile([128, NQ], F32, tag="en")
            nc.scalar.activation(
                en, gl, func=ACTF.Exp, bias=-gate_b, scale=-1.0
            )
            den = small.tile([128, NQ], F32, tag="den")
            nc.vector.tensor_scalar_add(den, en, 1.0)
            g4 = small.tile([128, NQ], F32, tag="g4")
            nc.vector.reciprocal(g4, den)
            cp4 = small.tile([128, NQ], F32, tag="cp4")
            # cp = (g-1)*r  (negative; Wv is negated so R = +(1-g)*retrieved)
            nc.vector.scalar_tensor_tensor(
                cp4, g4, -1.0, r, op0=ALU.add, op1=ALU.mult
            )
            st["g4"] = g4
            st["cp4"] = cp4

        def stats_bcast(t, st):
            """token-major -> slot-major transposes + PE broadcasts + E'."""
            E, g4, cp4 = st["E"], st["g4"], st["cp4"]
            # rows land at partition 0 (engine reads must start at 0/32/64,
            # so cp and g transpose apart)
            cpT_ps = psX.tile([2, NQ, 128], F32, tag="bc2")
            for q in range(NQ):
                nc.tensor.transpose(
                    cpT_ps[0:1, q, :], cp4[:, q : q + 1], identT_sb
                )
            cp_sm = work.tile([1, NQ, 128], F32R, tag="cp_sm")
            nc.scalar.copy(cp_sm, cpT_ps[0:1, :, :])
            gT_ps = psX.tile([NSLOT, TOK], F32, tag="bc64")
            gT_v = gT_ps[0:1, :].rearrange("p (a b) -> p a b", b=128)
            for q in range(NQ):
                nc.tensor.transpose(
                    gT_v[:, q, :], g4[:, q : q + 1], identT_sb
                )
            g_sm = work.tile([1, NQ, 128], F32R, tag="g_sm")
            nc.scalar.copy(g_sm, gT_v)
            cpb_ps = psX.tile([NSLOT, TOK], F32, tag="bc64")
            for q in range(NQ):
                nc.tensor.matmul(
                    cpb_ps[:, ts(q, 128)],
                    auxbc_sb[0:1, 0:NSLOT],
                    cp_sm[0:1, q, :],
                    start=True, stop=True,
                )
            Ep = work.tile([NSLOT, TOK], BF16, tag="Ep")
            nc.vector.tensor_mul(Ep, E.bitcast(F32), cpb_ps)
            gb_ps = psS.tile([128, TOK], F32, tag="S")
            for q in range(NQ):
                nc.tensor.matmul(
                    gb_ps[:, ts(q, 128)],
                    auxbc_sb[0:1, :],
                    g_sm[0:1, q, :],
                    start=True, stop=True,
                )
            # bf16 g: measured DVE mul bf16*bf16 (876ns) beats bf16*fp32 (1.6us)
            g_sb = work.tile([128, TOK], BF16, tag="g_sb")
            nc.scalar.copy(g_sb, gb_ps)
            st["Ep"] = Ep
            st["g_sb"] = g_sb

        def combine(t, st, mid1=None, mid2=None):
            """out = g*x + Ep@Wv per chunk.

            c<10: DVE mul -> PE accumulates identity@xg into the R bank ->
                  ACT drains the finished sum into o4.
            c>=10: gpsimd mul -> DVE adds xg+R straight from PSUM.
            """
            Ep, g_sb, xh_sb = st["Ep"], st["g_sb"], st["xh"]
            # gpsimd muls issued upfront; DVE muls just-in-time per group so
            # the next tile's Ep-mul (issued at mid1) lands early in DVE's
            # FIFO instead of behind all ten muls
            for c in range(10, 16):
                xg = xg2pool.tile([128, TOK], BF16, tag="xg2")
                nc.gpsimd.tensor_mul(xg, xh_sb[:, c, :], g_sb)
                st[f"xg{c}"] = xg
            for a in range(4):
                o4 = opool.tile([128, 4, TOK], BF16, tag="o4")
                for cc in range(4):
                    c = 4 * a + cc
                    if c < 10:
                        xg = xgpool.tile([128, TOK], BF16, tag="xg")
                        nc.vector.tensor_mul(xg, xh_sb[:, c, :], g_sb)
                        st[f"xg{c}"] = xg
                    xg = st[f"xg{c}"]
                    R_ps = psR.tile([128, TOK], F32, tag="R")
                    if c < 10:
                        nc.tensor.matmul(
                            R_ps, Wv_sb[:, c, :], Ep, start=True, stop=False
                        )
                        nc.tensor.matmul(
                            R_ps, identA_sb, xg, start=False, stop=True
                        )
                        nc.scalar.copy(o4[:, cc, :], R_ps)
                    else:
                        nc.tensor.matmul(
                            R_ps, Wv_sb[:, c, :], Ep, start=True, stop=True
                        )
                        nc.vector.tensor_add(o4[:, cc, :], xg, R_ps)
                nc.sync.dma_start(
                    out=outT_v[:, 4 * a : 4 * a + 4, ts(t, TOK)], in_=o4
                )
                if a == 0 and mid1 is not None:
                    mid1()

        # 3-deep software pipeline, stall-free issue order per iter t:
        #   DVE: chain(t+1) first (so PE transposes never wait mid-tile),
        #   PE:  R/adds(t) -> bcasts(t+1) -> S(t+3) -> sg/tok(t+2),
        #   DMA prefetch 4 ahead so S(t+3) finds its tile resident.
        states = {}
        for i in range(4):
            states[i] = phase_dma(i)
        s_phase(0, states[0])
        stats_front(0, states[0])
        stats_chain(0, states[0])
        stats_bcast(0, states[0])
        s_phase(1, states[1])
        stats_front(1, states[1])
        s_phase(2, states[2])
        stats_front(2, states[2])
        for t in range(NT):
            if t + 4 < NT:
                states[t + 4] = phase_dma(t + 4)
            n1 = states[t + 1] if t + 1 < NT else None
            n2 = states[t + 2] if t + 2 < NT else None
            if n1 is not None:
                stats_chain(t + 1, n1)
            combine(
                t,
                states[t],
                mid1=(lambda: stats_bcast(t + 1, n1)) if n1 else None,
                mid2=None,
            )
            if t + 3 < NT:
                s_phase(t + 3, states[t + 3])
            if n2 is not None and t + 2 >= 3:
                stats_front(t + 2, n2)
            del states[t]

    nc.compile()
    return nc


def _fold_weights(memory, key_w, value_w, gate_w):
    """Fold projections into the 64-slot table; bf16 hi/lo splits for scores."""
    bf16 = ml_dtypes.bfloat16
    mem = np.asarray(memory, np.float64)
    Ws = mem @ np.asarray(key_w, np.float64)  # [64, 2048]
    gx = np.asarray(gate_w, np.float64)[0, :DIM]
    G65 = np.concatenate([Ws, gx[None, :]], axis=0)  # [65, 2048]
    GT = np.ascontiguousarray(G65.T)  # [2048, 65]
    G1 = GT.astype(bf16)  # hi part incl gate column
    G2 = (GT - G1.astype(np.float64)).astype(bf16)  # lo part
    G2[:, NSLOT] = 0  # drop gate lo bits (gate tolerates hi-only)

    Wvf = mem @ np.asarray(value_w, np.float64).T  # [64, 2048]
    WvN = (-Wvf).astype(bf16)  # negated: R = +(1-g)*retrieved
    gvv = (Wvf @ np.asarray(gate_w, np.float64)[0, DIM:]).astype(np.float32)

    auxsg = np.zeros((NSLOT, 2), np.float32)
    auxsg[:, 0] = 1.0
    auxsg[:, 1] = gvv
    auxbc = np.ones((1, 128), np.float32)
    identT = np.eye(128, dtype=np.float32)
    identA = np.eye(128, dtype=np.float32).astype(bf16)
    return G1, G2, WvN, auxsg, auxbc, identT, identA


def _split_x(xb):
    """x [L, DIM] fp32 -> dim-major bf16 hi/lo [DIM, L]."""
    bf16 = ml_dtypes.bfloat16
    xT = np.ascontiguousarray(xb.T).astype(np.float32)
    xh = xT.astype(bf16)
    xl = (xT - xh.astype(np.float32)).astype(bf16)
    return xh, xl


def kernel(x, memory, key_w, value_w, gate_w, gate_b, _trace=False, _tmpdir=None):
    x = np.asarray(x, dtype=np.float32)
    G1, G2, WvN, auxsg, auxbc, identT, identA = _fold_weights(
        np.asarray(memory, np.float32),
        np.asarray(key_w, np.float32),
        np.asarray(value_w, np.float32),
        np.asarray(gate_w, np.float32),
    )
    nc = _build(float(np.asarray(gate_b).reshape(-1)[0]))
    in_maps = []
    for b in range(B):
        xh, xl = _split_x(x[b])
        in_maps.append(
            {
                "xh": xh, "xl": xl, "G1": G1, "G2": G2, "Wv": WvN,
                "auxsg": auxsg, "auxbc": auxbc, "identT": identT, "identA": identA,
            }
        )
    res = run_bass_kernel_spmd(
        nc, in_maps, list(range(B)), trace=_trace, tmpdir=_tmpdir
    )
    out = np.stack(
        [res.results[b]["outT"].astype(np.float32).T for b in range(B)], axis=0
    )
    if _trace:
        return out, res
    return out


# revision 40
# speedup vs baseline: 1.0654x; 1.0654x over previous
"""MemoryBank kernel for 8x TRN2 NeuronCores (SPMD, batch-parallel), v3.

Algebraic restructure (exact in real arithmetic):
    scores   = x @ (memory @ key_w).T            # fold key proj into 64-slot table
    gate_x   = x @ gate_w[:, :D].T               # 65th column of the folded table
    attn     = softmax(scores / 0.1)
    retrieved= attn @ (memory @ value_w.T)       # fold value proj into table
    ret_gate = attn @ (memory @ value_w.T @ gate_w[:, D:].T)
    gate     = sigmoid(gate_x + ret_gate + b)
    out      = gate * x + (1 - gate) * retrieved

Design (v3):
  * S matmul in split-bf16 (x = xh+xl, G = Gh+Gl; S = x@[Gh|gxh] + xh@Gl,
    dropping the 2^-18 xl@Gl term): full-rate bf16 instead of 4-cycle/row
    fp32 LOW_HIGH. Logit error ~2e-3 against sharp-softmax tolerance ~0.1.
  * Slot-major softmax: per-token max via gpsimd partition_all_reduce
    (result pre-broadcast over the 64 slot partitions), exp on ACT with the
    only table load of the kernel (sigmoid is computed as 1/(1+exp(-z))).
  * Per-token scalars (1/sum, gate, cp=(g-1)/sum) run token-major [128,4]
    via tiny f32r PE transposes (single-instruction, unlike fp32 LOW_HIGH),
    then broadcast back with 1-row f32r ones-matmuls.
  * Combine out = g*x + Ep@Wv: DVE does all 16 muls (bf16*fp32->bf16) and
    4 PSUM-direct adds; ACT drains 12 R-tiles to SBUF bf16 for gpsimd adds
    (gpsimd cannot touch PSUM).
  * bf16 output: 48MiB HBM traffic per core (~140us roofline at 358GB/s).
  * 3-deep software pipeline: combine(t) | stats_back(t+1) | stats_front(t+2)
    | S(t+3) | DMA(t+3) so the ~2-cadence stats latency never blocks combine.
"""

from contextlib import ExitStack

import numpy as np
import ml_dtypes

import concourse.bass as bass
import concourse.tile as tile
from concourse import bacc
from concourse import bass_isa
from concourse import mybir
from concourse.bass import ts
from concourse.bass_utils import run_bass_kernel_spmd

F32 = mybir.dt.float32
F32R = mybir.dt.float32r
BF16 = mybir.dt.bfloat16
AX = mybir.AxisListType
ALU = mybir.AluOpType
ACTF = mybir.ActivationFunctionType

B = 8
L = 4096
DIM = 2048
NSLOT = 64
NCH = DIM // 128  # 16 dim chunks
TOK = 512  # tokens per tile
NT = L // TOK  # 8 tiles per core
NQ = TOK // 128  # 4 token quarters per tile

N_DVE_ADD = 4  # combine adds on DVE (PSUM-direct); rest drained for gpsimd


def _build(gate_b: float) -> bass.Bass:
    nc = bacc.Bacc("TRN2", target_bir_lowering=False, debug=False)

    xh = nc.dram_tensor("xh", [DIM, L], BF16, kind="ExternalInput").ap()
    xl = nc.dram_tensor("xl", [DIM, L], BF16, kind="ExternalInput").ap()
    G1 = nc.dram_tensor("G1", [DIM, NSLOT + 1], BF16, kind="ExternalInput").ap()
    G2 = nc.dram_tensor("G2", [DIM, NSLOT + 1], BF16, kind="ExternalInput").ap()
    Wv = nc.dram_tensor("Wv", [NSLOT, DIM], BF16, kind="ExternalInput").ap()
    auxsg = nc.dram_tensor("auxsg", [NSLOT, 2], F32R, kind="ExternalInput").ap()
    auxbc = nc.dram_tensor("auxbc", [1, 128], F32R, kind="ExternalInput").ap()
    identT = nc.dram_tensor("identT", [128, 128], F32, kind="ExternalInput").ap()
    identA = nc.dram_tensor("identA", [128, 128], BF16, kind="ExternalInput").ap()
    outT = nc.dram_tensor("outT", [DIM, L], BF16, kind="ExternalOutput").ap()

    # dim d = c*128 + p  (chunk-major split; consistent everywhere)
    xh_v = xh.rearrange("(c p) t -> p c t", p=128)
    xl_v = xl.rearrange("(c p) t -> p c t", p=128)
    G1_v = G1.rearrange("(c p) m -> p c m", p=128)
    G2_v = G2.rearrange("(c p) m -> p c m", p=128)
    Wv_v = Wv.rearrange("n (c q) -> n c q", q=128)
    outT_v = outT.rearrange("(c p) t -> p c t", p=128)

    with tile.TileContext(nc) as tc, ExitStack() as ctx:
        consts = ctx.enter_context(tc.tile_pool(name="consts", bufs=1))
        xhpool = ctx.enter_context(tc.tile_pool(name="xhpool", bufs=5))
        xlpool = ctx.enter_context(tc.tile_pool(name="xlpool", bufs=2))
        opool = ctx.enter_context(tc.tile_pool(name="opool", bufs=2))
        work = ctx.enter_context(tc.tile_pool(name="work", bufs=3))
        small = ctx.enter_context(tc.tile_pool(name="small", bufs=2))
        xgpool = ctx.enter_context(tc.tile_pool(name="xgpool", bufs=10))
        xg2pool = ctx.enter_context(tc.tile_pool(name="xg2pool", bufs=6))
        psS = ctx.enter_context(tc.tile_pool(name="psS", bufs=2, space="PSUM"))
        psR = ctx.enter_context(tc.tile_pool(name="psR", bufs=2, space="PSUM"))
        # single-buffered aux banks; same-shape tags reuse a bank across the
        # serial stats chain (WAR hazards order them)
        psX = ctx.enter_context(tc.tile_pool(name="psX", bufs=1, space="PSUM"))

        G1_sb = consts.tile([128, NCH, NSLOT + 1], BF16)
        nc.sync.dma_start(out=G1_sb, in_=G1_v)
        G2_sb = consts.tile([128, NCH, NSLOT + 1], BF16)
        nc.sync.dma_start(out=G2_sb, in_=G2_v)
        Wv_sb = consts.tile([NSLOT, NCH, 128], BF16)
        nc.sync.dma_start(out=Wv_sb, in_=Wv_v)
        auxsg_sb = consts.tile([NSLOT, 2], F32R)
        nc.sync.dma_start(out=auxsg_sb, in_=auxsg)
        auxbc_sb = consts.tile([1, 128], F32R)
        nc.sync.dma_start(out=auxbc_sb, in_=auxbc)
        identT_sb = consts.tile([128, 128], F32)
        nc.sync.dma_start(out=identT_sb, in_=identT)
        identA_sb = consts.tile([128, 128], BF16)
        nc.sync.dma_start(out=identA_sb, in_=identA)

        def phase_dma(t):
            """prefetch x^T hi/lo tiles (3 tiles ahead of use)."""
            xh_sb = xhpool.tile([128, NCH, TOK], BF16, tag="xh_sb")
            nc.sync.dma_start(out=xh_sb, in_=xh_v[:, :, ts(t, TOK)])
            xl_sb = xlpool.tile([128, NCH, TOK], BF16, tag="xl_sb")
            nc.sync.dma_start(out=xl_sb, in_=xl_v[:, :, ts(t, TOK)])
            return {"xh": xh_sb, "xl": xl_sb}

        def s_phase(t, st):
            """split-bf16 score matmuls + PSUM->SBUF copy.

            S = x@[Gh|gxh] + xh@[Gl|0]  (rows 0..63 slots, row 64 gate x-part)
            """
            xh_sb, xl_sb = st["xh"], st["xl"]
            S_ps = psS.tile([NSLOT + 1, TOK], F32, tag="S")
            for c in range(NCH):
                nc.tensor.matmul(
                    S_ps, G1_sb[:, c, :], xh_sb[:, c, :],
                    start=(c == 0), stop=False,
                )
                nc.tensor.matmul(
                    S_ps, G1_sb[:, c, :], xl_sb[:, c, :],
                    start=False, stop=False,
                )
                nc.tensor.matmul(
                    S_ps, G2_sb[:, c, :], xh_sb[:, c, :],
                    start=False, stop=(c == NCH - 1),
                )
            S_sb = work.tile([NSLOT + 1, TOK], F32, tag="S_sb")
            nc.scalar.copy(S_sb, S_ps)
            st["S_sb"] = S_sb

        def stats_front(t, st):
            """max (gpsimd, pre-broadcast), exp, slot sums, token-major stage."""
            S_sb = st["S_sb"]
            mxa = small.tile([NSLOT, TOK], F32, tag="mxa")
            nc.gpsimd.partition_all_reduce(
                mxa, S_sb[0:NSLOT, :], channels=NSLOT,
                reduce_op=bass_isa.ReduceOp.max,
            )
            Ssub = work.tile([NSLOT, TOK], F32, tag="Ssub")
            nc.vector.tensor_sub(Ssub, S_sb[0:NSLOT, :], mxa)
            # F32R so the sums/gvd matmul may consume it directly
            E = work.tile([NSLOT, TOK], F32R, tag="E")
            nc.scalar.activation(E, Ssub, func=ACTF.Exp, bias=0.0, scale=10.0)
            st["E"] = E
            sg_ps = psX.tile([2, NQ, 128], F32, tag="bc2")
            sg_flat = sg_ps.rearrange("p a b -> p (a b)")
            nc.tensor.matmul(sg_flat, auxsg_sb, E, start=True, stop=True)
            sgx_sb = work.tile([2, TOK], F32, tag="sgx_sb")
            nc.scalar.copy(sgx_sb, sg_flat)
            gx_sb = work.tile([1, TOK], F32, tag="gx_sb")
            nc.scalar.copy(gx_sb, S_sb[NSLOT : NSLOT + 1, :])
            tok_ps = psX.tile([128, NQ, 3], F32, tag="tok")
            for q in range(NQ):
                nc.tensor.transpose(
                    tok_ps[:, q, 0:2], sgx_sb[:, ts(q, 128)],
                    identT_sb[0:2, 0:2],
                )
                nc.tensor.transpose(
                    tok_ps[:, q, 2:3], gx_sb[:, ts(q, 128)],
                    identT_sb[0:1, 0:1],
                )
            st["tok"] = tok_ps

        def stats_chain(t, st):
            """token-major scalar chain (DVE+ACT only; issued at iter start
            so the PE's transposes never wait on it mid-tile)."""
            tok_ps = st["tok"]
            r = small.tile([128, NQ], F32, tag="r")
            nc.vector.reciprocal(r, tok_ps[:, :, 0])
            t1 = small.tile([128, NQ], F32, tag="t1")
            nc.vector.tensor_mul(t1, tok_ps[:, :, 1], r)
            gl = small.tile([128, NQ], F32, tag="gl")
            nc.vector.tensor_add(gl, t1, tok_ps[:, :, 2])
            # sigmoid(z) = 1/(1+exp(-z)) on the Exp table (no table thrash)
            en = small.tile([128, NQ], F32, tag="en")
            nc.scalar.activation(
                en, gl, func=ACTF.Exp, bias=-gate_b, scale=-1.0
            )
            den = small.tile([128, NQ], F32, tag="den")
            nc.vector.tensor_scalar_add(den, en, 1.0)
            g4 = small.tile([128, NQ], F32, tag="g4")
            nc.vector.reciprocal(g4, den)
            cp4 = small.tile([128, NQ], F32, tag="cp4")
            # cp = (g-1)*r  (negative; Wv is negated so R = +(1-g)*retrieved)
            nc.vector.scalar_tensor_tensor(
                cp4, g4, -1.0, r, op0=ALU.add, op1=ALU.mult
            )
            st["g4"] = g4
            st["cp4"] = cp4

        def stats_bcast(t, st):
            """token-major -> slot-major transposes + PE broadcasts + E'."""
            E, g4, cp4 = st["E"], st["g4"], st["cp4"]
            # rows land at partition 0 (engine reads must start at 0/32/64,
            # so cp and g transpose apart)
            cpT_ps = psX.tile([2, NQ, 128], F32, tag="bc2")
            for q in range(NQ):
                nc.tensor.transpose(
                    cpT_ps[0:1, q, :], cp4[:, q : q + 1], identT_sb
                )
            cp_sm = work.tile([1, NQ, 128], F32R, tag="cp_sm")
            nc.scalar.copy(cp_sm, cpT_ps[0:1, :, :])
            gT_ps = psX.tile([NSLOT, TOK], F32, tag="bc64")
            gT_v = gT_ps[0:1, :].rearrange("p (a b) -> p a b", b=128)
            for q in range(NQ):
                nc.tensor.transpose(
                    gT_v[:, q, :], g4[:, q : q + 1], identT_sb
                )
            g_sm = work.tile([1, NQ, 128], F32R, tag="g_sm")
            nc.scalar.copy(g_sm, gT_v)
            cpb_ps = psX.tile([NSLOT, TOK], F32, tag="bc64")
            for q in range(NQ):
                nc.tensor.matmul(
                    cpb_ps[:, ts(q, 128)],
                    auxbc_sb[0:1, 0:NSLOT],
                    cp_sm[0:1, q, :],
                    start=True, stop=True,
                )
            Ep = work.tile([NSLOT, TOK], BF16, tag="Ep")
            nc.vector.tensor_mul(Ep, E.bitcast(F32), cpb_ps)
            gb_ps = psX.tile([128, TOK], F32, tag="gb")
            for q in range(NQ):
                nc.tensor.matmul(
                    gb_ps[:, ts(q, 128)],
                    auxbc_sb[0:1, :],
                    g_sm[0:1, q, :],
                    start=True, stop=True,
                )
            # bf16 g: measured DVE mul bf16*bf16 (876ns) beats bf16*fp32 (1.6us)
            g_sb = work.tile([128, TOK], BF16, tag="g_sb")
            nc.scalar.copy(g_sb, gb_ps)
            st["Ep"] = Ep
            st["g_sb"] = g_sb

        def combine(t, st, mid1=None, mid2=None):
            """out = g*x + Ep@Wv per chunk.

            c<10: DVE mul -> PE accumulates identity@xg into the R bank ->
                  ACT drains the finished sum into o4.
            c>=10: gpsimd mul -> DVE adds xg+R straight from PSUM.
            """
            Ep, g_sb, xh_sb = st["Ep"], st["g_sb"], st["xh"]
            # gpsimd muls issued upfront; DVE muls just-in-time per group so
            # the next tile's Ep-mul (issued at mid1) lands early in DVE's
            # FIFO instead of behind all ten muls
            for c in range(10, 16):
                xg = xg2pool.tile([128, TOK], BF16, tag="xg2")
                nc.gpsimd.tensor_mul(xg, xh_sb[:, c, :], g_sb)
                st[f"xg{c}"] = xg
            for a in range(4):
                o4 = opool.tile([128, 4, TOK], BF16, tag="o4")
                for cc in range(4):
                    c = 4 * a + cc
                    if c < 10:
                        xg = xgpool.tile([128, TOK], BF16, tag="xg")
                        nc.vector.tensor_mul(xg, xh_sb[:, c, :], g_sb)
                        st[f"xg{c}"] = xg
                    xg = st[f"xg{c}"]
                    R_ps = psR.tile([128, TOK], F32, tag="R")
                    if c < 10:
                        nc.tensor.matmul(
                            R_ps, Wv_sb[:, c, :], Ep, start=True, stop=False
                        )
                        nc.tensor.matmul(
                            R_ps, identA_sb, xg, start=False, stop=True
                        )
                        nc.scalar.copy(o4[:, cc, :], R_ps)
                    else:
                        nc.tensor.matmul(
                            R_ps, Wv_sb[:, c, :], Ep, start=True, stop=True
                        )
                        nc.vector.tensor_add(o4[:, cc, :], xg, R_ps)
                nc.sync.dma_start(
                    out=outT_v[:, 4 * a : 4 * a + 4, ts(t, TOK)], in_=o4
                )
                if a == 0 and mid1 is not None:
                    mid1()

        # 3-deep software pipeline, stall-free issue order per iter t:
        #   DVE: chain(t+1) first (so PE transposes never wait mid-tile),
        #   PE:  R/adds(t) -> bcasts(t+1) -> S(t+3) -> sg/tok(t+2),
        #   DMA prefetch 4 ahead so S(t+3) finds its tile resident.
        states = {}
        for i in range(4):
            states[i] = phase_dma(i)
        s_phase(0, states[0])
        stats_front(0, states[0])
        stats_chain(0, states[0])
        stats_bcast(0, states[0])
        s_phase(1, states[1])
        stats_front(1, states[1])
        s_phase(2, states[2])
        stats_front(2, states[2])
        for t in range(NT):
            if t + 4 < NT:
                states[t + 4] = phase_dma(t + 4)
            n1 = states[t + 1] if t + 1 < NT else None
            n2 = states[t + 2] if t + 2 < NT else None
            if n1 is not None:
                stats_chain(t + 1, n1)
            combine(
                t,
                states[t],
                mid1=(lambda: stats_bcast(t + 1, n1)) if n1 else None,
                mid2=None,
            )
            if t + 3 < NT:
                s_phase(t + 3, states[t + 3])
            if n2 is not None and t + 2 >= 3:
                stats_front(t + 2, n2)
            del states[t]

    nc.compile()
    return nc


def _fold_weights(memory, key_w, value_w, gate_w):
    """Fold projections into the 64-slot table; bf16 hi/lo splits for scores."""
    bf16 = ml_dtypes.bfloat16
    mem = np.asarray(memory, np.float64)
    Ws = mem @ np.asarray(key_w, np.float64)  # [64, 2048]
    gx = np.asarray(gate_w, np.float64)[0, :DIM]
    G65 = np.concatenate([Ws, gx[None, :]], axis=0)  # [65, 2048]
    GT = np.ascontiguousarray(G65.T)  # [2048, 65]
    G1 = GT.astype(bf16)  # hi part incl gate column
    G2 = (GT - G1.astype(np.float64)).astype(bf16)  # lo part
    G2[:, NSLOT] = 0  # drop gate lo bits (gate tolerates hi-only)

    Wvf = mem @ np.asarray(value_w, np.float64).T  # [64, 2048]
    WvN = (-Wvf).astype(bf16)  # negated: R = +(1-g)*retrieved
    gvv = (Wvf @ np.asarray(gate_w, np.float64)[0, DIM:]).astype(np.float32)

    auxsg = np.zeros((NSLOT, 2), np.float32)
    auxsg[:, 0] = 1.0
    auxsg[:, 1] = gvv
    auxbc = np.ones((1, 128), np.float32)
    identT = np.eye(128, dtype=np.float32)
    identA = np.eye(128, dtype=np.float32).astype(bf16)
    return G1, G2, WvN, auxsg, auxbc, identT, identA


def _split_x(xb):
    """x [L, DIM] fp32 -> dim-major bf16 hi/lo [DIM, L]."""
    bf16 = ml_dtypes.bfloat16
    xT = np.ascontiguousarray(xb.T).astype(np.float32)
    xh = xT.astype(bf16)
    xl = (xT - xh.astype(np.float32)).astype(bf16)
    return xh, xl


def kernel(x, memory, key_w, value_w, gate_w, gate_b, _trace=False, _tmpdir=None):
    x = np.asarray(x, dtype=np.float32)
    G1, G2, WvN, auxsg, auxbc, identT, identA = _fold_weights(
        np.asarray(memory, np.float32),
        np.asarray(key_w, np.float32),
        np.asarray(value_w, np.float32),
        np.asarray(gate_w, np.float32),
    )
    nc = _build(float(np.asarray(gate_b).reshape(-1)[0]))
    in_maps = []
    for b in range(B):
        xh, xl = _split_x(x[b])
        in_maps.append(
            {
                "xh": xh, "xl": xl, "G1": G1, "G2": G2, "Wv": WvN,
                "auxsg": auxsg, "auxbc": auxbc, "identT": identT, "identA": identA,
            }
        )
    res = run_bass_kernel_spmd(
        nc, in_maps, list(range(B)), trace=_trace, tmpdir=_tmpdir
    )
    out = np.stack(
        [res.results[b]["outT"].astype(np.float32).T for b in range(B)], axis=0
    )
    if _trace:
        return out, res
    return out
